# revision 6
# baseline (speedup 1.0000x reference)
"""KNN top-16 kernel for Trainium2 (8 NeuronCores, SPMD) — v2.

Problem (hardcoded): p1 (4,8192,3) f32, p2 (4,8192,3) f32, lengths1/2 (4,) i32.
Returns (idx int64 (4,8192,16), dists f32 (4,8192,16)) matching
jax.lax.top_k(-sq_dists, 16) semantics with PyTorch3D-style padding.

Sharding: core c handles batch n=c//2, query rows [(c%2)*4096, (c%2+1)*4096).
p2 of that batch is replicated to the core.

v2 design (vs baseline's 6 full DVE passes):
  score s[i,j] = 2*p1_i.p2_j - ||p2_j||^2 - BIG*(j >= len2), computed by a
  single 22-contraction-row bf16 matmul (3-way bf16 split of both operands:
  6 cross terms per dim + 3 rows for the fp32-split ||p2||^2 + 1 mask row),
  accurate to ~fp32 level but streaming 4x faster than fp32 on the PE.

  Top-16 per query row via per-chunk candidates: for each 512-wide chunk the
  DVE reads the PSUM bank directly (no SBUF copy): max8 -> top-8 values,
  max_index -> their local indices.  Top-8 per 512-chunk provably contains
  the global top-16 unless >8 of the top-16 land in one chunk (verified: 0
  such rows in this dataset, worst count 7/8).  A cheap 128-wide final pass
  (max8, max_index, match_replace, max8, max_index) extracts the top-16
  values + candidate positions.  Host maps positions -> global indices via
  the candidate local-index array and recomputes dists exactly by gathering.

  DVE work drops from ~6 to ~2 passes over the 33M scores; the PSUM->SBUF
  copy and full-width match_replace disappear entirely.
"""

import numpy as np
from functools import lru_cache

N, P1, P2, D, K = 4, 8192, 8192, 3, 16
N_CORES = 8
QPC = P1 // 2          # queries per core (4096)
TILE = 128             # query rows per tile
NTILES = QPC // TILE   # 32
CHUNK = 512            # matmul free-dim chunk == one PSUM bank
NCHUNK = P2 // CHUNK   # 16
ROWS = 22              # contraction rows
BIG = np.float32(1e30)
INW = QPC + P2         # packed input width per partition (12288)
NCAND = NCHUNK * 8     # candidates per tile (128)
USE_SCALAR_COPY = False  # PSUM-direct DVE reads measure faster than SBUF-staged


@lru_cache(maxsize=1)
def _build_program():
    from concourse.bass import Bass
    from concourse.tile import TileContext
    import concourse.mybir as mybir

    f32 = mybir.dt.float32
    bf16 = mybir.dt.bfloat16
    u32 = mybir.dt.uint32

    nc = Bass("TRN2", num_devices=N_CORES)

    inp_d = nc.dram_tensor("inp", [ROWS, INW], bf16, kind="ExternalInput")
    # per-tile candidates: top-8 values + local indices per 512-chunk
    cv_d = nc.dram_tensor("cv_out", [TILE, NTILES * NCAND], f32,
                          kind="ExternalOutput")
    cidx_d = nc.dram_tensor("cidx_out", [TILE, NTILES * NCAND], u32,
                            kind="ExternalOutput")

    with TileContext(nc) as tc:
        with tc.tile_pool(name="const", bufs=1) as cpool, \
             tc.tile_pool(name="chunk", bufs=8) as kpool, \
             tc.tile_pool(name="psum", bufs=8, space="PSUM") as ppool:
            inp_sb = cpool.tile([ROWS, INW], bf16)
            nc.sync.dma_start(inp_sb, inp_d[:, :])
            lhs_sb = inp_sb[:, 0:QPC]
            rhs_sb = inp_sb[:, QPC:INW]

            # Persistent result staging: each region written exactly once, so
            # DVE writes carry no slot-reuse deps; two DMAs at the end.
            cand_v = cpool.tile([TILE, NTILES * NCAND], f32)
            cand_i = cpool.tile([TILE, NTILES * NCAND], u32)

            for t in range(NTILES):
                lhsT = lhs_sb[:, t * TILE:(t + 1) * TILE]
                for c in range(NCHUNK):
                    ps = ppool.tile([TILE, CHUNK], f32, tag="ps")
                    nc.tensor.matmul(
                        ps, lhsT, rhs_sb[:, c * CHUNK:(c + 1) * CHUNK],
                        start=True, stop=True,
                    )
                    if USE_SCALAR_COPY:
                        ck = kpool.tile([TILE, CHUNK], f32, tag="ck")
                        nc.scalar.copy(ck, ps)
                        src = ck
                    else:
                        src = ps
                    base = (t * NCHUNK + c) * 8
                    cv = cand_v[:, base:base + 8]
                    nc.vector.max(out=cv, in_=src)
                    nc.vector.max_index(
                        out=cand_i[:, base:base + 8], in_max=cv, in_values=src)

            nc.sync.dma_start(cv_d[:, :], cand_v)
            nc.sync.dma_start(cidx_d[:, :], cand_i)

    # This walrus build allows only ~1 sync wait per instruction; split all
    # but the last wait onto single-wait NoOps chained before it (same
    # engine, program order => identical blocking semantics).
    import concourse.mybir as mb
    fix = 0
    for fn in nc.m.functions:
        for blk in fn.blocks:
            insts = blk.instructions
            i = 0
            while i < len(insts):
                inst = insts[i]
                si = inst.sync_info
                if si is not None and len(si.on_wait) > 1:
                    head, last = si.on_wait[:-1], si.on_wait[-1:]
                    pre = []
                    for w in head:
                        fix += 1
                        nop = mb.InstNoOp(name=f"I-waitfix-{fix}", ins=[],
                                          outs=[])
                        nop.engine = inst.engine
                        nop.sync_info = mb.SyncInfo(on_wait=[w], on_update=[])
                        pre.append(nop)
                    si.on_wait = last
                    insts[i:i] = pre
                    i += len(pre)
                i += 1
    return nc


def _split3(x):
    """3-way bf16 split: x ~= h + l1 + l2 (all bf16), error ~2^-27 |x|."""
    import ml_dtypes
    bf = ml_dtypes.bfloat16
    x = np.asarray(x, np.float32)
    h = x.astype(bf)
    l1 = (x - h.astype(np.float32)).astype(bf)
    l2 = (x - h.astype(np.float32) - l1.astype(np.float32)).astype(bf)
    return h, l1, l2


def _core_inputs(p1, p2, lengths2, core):
    import ml_dtypes
    bf = ml_dtypes.bfloat16
    n, h = core // 2, core % 2
    q0 = h * QPC
    p1n = p1[n, q0:q0 + QPC]          # (4096, 3)
    p2n = p2[n]                        # (8192, 3)

    inp = np.zeros((ROWS, INW), bf)
    lhs = inp[:, 0:QPC]
    rhs = inp[:, QPC:INW]
    r = 0
    for d in range(D):
        A0, A1, A2 = _split3(p1n[:, d])
        B0, B1, B2 = _split3(p2n[:, d])
        A0f, A1f, A2f = (a.astype(np.float32) for a in (A0, A1, A2))
        # terms (lhs carries the 2x; exact in bf16 since *2 bumps the exponent)
        for a, b in [(A0f, B0), (A0f, B1), (A0f, B2),
                     (A1f, B0), (A2f, B0), (A1f, B1)]:
            lhs[r] = (2.0 * a).astype(bf)
            rhs[r] = b
            r += 1
    p2sq = (p2n.astype(np.float32) ** 2).sum(axis=1, dtype=np.float32)
    for s in _split3(p2sq):
        lhs[r] = bf(-1.0)
        rhs[r] = s
        r += 1
    lhs[r] = bf(1.0)
    rhs[r] = np.where(np.arange(P2) >= lengths2[n], -BIG,
                      np.float32(0.0)).astype(bf)
    r += 1
    assert r == ROWS
    return {"inp": inp}


def kernel(p1, p2, lengths1, lengths2):
    from concourse.bass_utils import run_bass_kernel_spmd

    p1 = np.asarray(p1, np.float32)
    p2 = np.asarray(p2, np.float32)
    lengths1 = np.asarray(lengths1, np.int32)
    lengths2 = np.asarray(lengths2, np.int32)

    nc = _build_program()
    in_maps = [_core_inputs(p1, p2, lengths2, c) for c in range(N_CORES)]
    res = run_bass_kernel_spmd(nc, in_maps, core_ids=list(range(N_CORES)))

    # host epilogue: decode candidate positions -> global indices, then
    # recompute dists exactly (same fp32 formula as the reference).
    p1sq = np.sum(p1 * p1, axis=2, dtype=np.float32)    # (4, 8192)
    p2sq = np.sum(p2 * p2, axis=2, dtype=np.float32)    # (4, 8192)

    dists = np.zeros((N, P1, K), np.float32)
    idx = np.zeros((N, P1, K), np.int64)
    for c in range(N_CORES):
        n, h = c // 2, c % 2
        sl = slice(h * QPC, (h + 1) * QPC)
        # query-major candidate arrays: (QPC, 128)
        cv = (res.results[c]["cv_out"].reshape(TILE, NTILES, NCAND)
              .transpose(1, 0, 2).reshape(QPC, NCAND))
        ci = (res.results[c]["cidx_out"].reshape(TILE, NTILES, NCAND)
              .transpose(1, 0, 2).reshape(QPC, NCAND))
        # final top-16 of the 128 candidates (desc value, ties -> lower pos)
        part = np.argpartition(-cv, K - 1, axis=1)[:, :K]
        pv = np.take_along_axis(cv, part, axis=1)
        ordr = np.lexsort((part, -pv), axis=1)
        pos = np.take_along_axis(part, ordr, axis=1)     # (QPC, 16) cand pos
        local = np.take_along_axis(ci, pos, axis=1).astype(np.int64)
        j = (pos >> 3) * CHUNK + local                   # global p2 indices
        # exact dists via gather
        g = p2[n][j]                                     # (4096, 16, 3)
        dots = np.einsum("pd,pkd->pk", p1[n, sl], g)
        dists[n, sl] = p1sq[n, sl, None] + p2sq[n][j] - 2.0 * dots
        idx[n, sl] = j

    for n in range(N):
        L = int(lengths1[n])
        dists[n, L:] = 0.0
        idx[n, L:] = 0
    return idx, dists


# revision 7
# speedup vs baseline: 1.0952x; 1.0952x over previous
"""KNN top-16 kernel for Trainium2 (8 NeuronCores, SPMD) — v3.

Problem (hardcoded): p1 (4,8192,3) f32, p2 (4,8192,3) f32, lengths1/2 (4,) i32.
Returns (idx int64 (4,8192,16), dists f32 (4,8192,16)) matching
jax.lax.top_k(-sq_dists, 16) semantics with PyTorch3D-style padding.

Sharding: core c handles batch n=c//2, query rows [(c%2)*4096, (c%2+1)*4096);
p2 of that batch is replicated to the core.

Device algorithm (per 128-query tile):
  score s[i,j] = 2*p1_i.p2_j - ||p2_j||^2 - BIG*(j >= len2), via a single
  22-contraction-row bf16 matmul (3-way bf16 split of both operands: 6 cross
  terms per dim + 3 rows for the fp32-split ||p2||^2 + 1 mask row) —
  ~fp32-accurate but 4x faster PE streaming than fp32.

  Per 1024-wide window the DVE reads the PSUM pair-bank directly:
  max8 -> top-8 values, max_index -> local indices.  8 window op-pairs per
  tile instead of 16 (vs 512-chunks) cuts DVE instruction overheads.

Host epilogue:
  top-16 of the 64 candidates per row (value-desc, position-stable),
  global index = window*1024 + local, dists recomputed exactly by gathering.
  Exactness guard: a window's top-8 cannot cover >8 of a row's true top-16;
  any row where one window contributes all 8 of its candidates to the final
  top-16 *might* have lost a 9th - such rows (~1k of 32k) are recomputed
  exactly on the host (complete detection, so the result is exact).
"""

import numpy as np
from functools import lru_cache

N, P1, P2, D, K = 4, 8192, 8192, 3, 16
N_CORES = 8
QPC = P1 // 2          # queries per core (4096)
TILE = 128             # query rows per tile
NTILES = QPC // TILE   # 32
WIN = 1024             # top-8 window width == one 2-bank PSUM tile
NWIN = P2 // WIN       # 8
MMN = 512              # matmul free-dim per instruction
ROWS = 22              # contraction rows
BIG = np.float32(1e30)
INW = QPC + P2         # packed input width per partition (12288)
NCAND = NWIN * 8       # candidates per tile (64)
OUT_SPLIT = 4          # output DMA pieces (overlap with compute)


@lru_cache(maxsize=1)
def _build_program():
    from concourse.bass import Bass
    from concourse.tile import TileContext
    import concourse.mybir as mybir

    f32 = mybir.dt.float32
    bf16 = mybir.dt.bfloat16
    u32 = mybir.dt.uint32

    nc = Bass("TRN2", num_devices=N_CORES)

    inp_d = nc.dram_tensor("inp", [ROWS, INW], bf16, kind="ExternalInput")
    # per-tile candidates: top-8 values + local indices per 1024-window
    cv_d = nc.dram_tensor("cv_out", [TILE, NTILES * NCAND], f32,
                          kind="ExternalOutput")
    cidx_d = nc.dram_tensor("cidx_out", [TILE, NTILES * NCAND], u32,
                            kind="ExternalOutput")

    with TileContext(nc) as tc:
        with tc.tile_pool(name="const", bufs=1) as cpool, \
             tc.tile_pool(name="psum", bufs=4, space="PSUM") as ppool:
            inp_sb = cpool.tile([ROWS, INW], bf16)
            # split the input DMA so the first matmuls start sooner
            HEAD = QPC + 2 * WIN
            nc.sync.dma_start(inp_sb[:, 0:HEAD], inp_d[:, 0:HEAD])
            nc.sync.dma_start(inp_sb[:, HEAD:INW], inp_d[:, HEAD:INW])
            lhs_sb = inp_sb[:, 0:QPC]
            rhs_sb = inp_sb[:, QPC:INW]

            # Persistent result staging: each region written exactly once, so
            # DVE writes carry no slot-reuse deps.
            cand_v = cpool.tile([TILE, NTILES * NCAND], f32)
            cand_i = cpool.tile([TILE, NTILES * NCAND], u32)

            tiles_per_piece = NTILES // OUT_SPLIT
            for t in range(NTILES):
                lhsT = lhs_sb[:, t * TILE:(t + 1) * TILE]
                for w in range(NWIN):
                    ps = ppool.tile([TILE, WIN], f32, tag="ps")
                    for h in range(WIN // MMN):
                        c0 = w * WIN + h * MMN
                        nc.tensor.matmul(
                            ps[:, h * MMN:(h + 1) * MMN], lhsT,
                            rhs_sb[:, c0:c0 + MMN],
                            start=True, stop=True,
                        )
                    base = (t * NWIN + w) * 8
                    cv = cand_v[:, base:base + 8]
                    nc.vector.max(out=cv, in_=ps)
                    nc.vector.max_index(
                        out=cand_i[:, base:base + 8], in_max=cv, in_values=ps)
                if (t + 1) % tiles_per_piece == 0:
                    a = (t + 1 - tiles_per_piece) * NCAND
                    b = (t + 1) * NCAND
                    nc.sync.dma_start(cv_d[:, a:b], cand_v[:, a:b])
                    nc.sync.dma_start(cidx_d[:, a:b], cand_i[:, a:b])

    # This walrus build allows only ~1 sync wait per instruction; split all
    # but the last wait onto single-wait NoOps chained before it (same
    # engine, program order => identical blocking semantics).
    import concourse.mybir as mb
    fix = 0
    for fn in nc.m.functions:
        for blk in fn.blocks:
            insts = blk.instructions
            i = 0
            while i < len(insts):
                inst = insts[i]
                si = inst.sync_info
                if si is not None and len(si.on_wait) > 1:
                    head, last = si.on_wait[:-1], si.on_wait[-1:]
                    pre = []
                    for w in head:
                        fix += 1
                        nop = mb.InstNoOp(name=f"I-waitfix-{fix}", ins=[],
                                          outs=[])
                        nop.engine = inst.engine
                        nop.sync_info = mb.SyncInfo(on_wait=[w], on_update=[])
                        pre.append(nop)
                    si.on_wait = last
                    insts[i:i] = pre
                    i += len(pre)
                i += 1
    return nc


def _split3(x):
    """3-way bf16 split: x ~= h + l1 + l2 (all bf16), error ~2^-27 |x|."""
    import ml_dtypes
    bf = ml_dtypes.bfloat16
    x = np.asarray(x, np.float32)
    h = x.astype(bf)
    l1 = (x - h.astype(np.float32)).astype(bf)
    l2 = (x - h.astype(np.float32) - l1.astype(np.float32)).astype(bf)
    return h, l1, l2


def _core_inputs(p1, p2, lengths2, core):
    import ml_dtypes
    bf = ml_dtypes.bfloat16
    n, h = core // 2, core % 2
    q0 = h * QPC
    p1n = p1[n, q0:q0 + QPC]          # (4096, 3)
    p2n = p2[n]                        # (8192, 3)

    inp = np.zeros((ROWS, INW), bf)
    lhs = inp[:, 0:QPC]
    rhs = inp[:, QPC:INW]
    r = 0
    for d in range(D):
        A0, A1, A2 = _split3(p1n[:, d])
        B0, B1, B2 = _split3(p2n[:, d])
        A0f, A1f, A2f = (a.astype(np.float32) for a in (A0, A1, A2))
        # terms (lhs carries the 2x; exact in bf16 since *2 bumps the exponent)
        for a, b in [(A0f, B0), (A0f, B1), (A0f, B2),
                     (A1f, B0), (A2f, B0), (A1f, B1)]:
            lhs[r] = (2.0 * a).astype(bf)
            rhs[r] = b
            r += 1
    p2sq = (p2n.astype(np.float32) ** 2).sum(axis=1, dtype=np.float32)
    for s in _split3(p2sq):
        lhs[r] = bf(-1.0)
        rhs[r] = s
        r += 1
    lhs[r] = bf(1.0)
    rhs[r] = np.where(np.arange(P2) >= lengths2[n], -BIG,
                      np.float32(0.0)).astype(bf)
    r += 1
    assert r == ROWS
    return {"inp": inp}


def _exact_rows(p1n, p2n, len2, rows):
    """Exact top-16 (reference fp32 formula) for the given query rows of one
    batch. Returns idx (len(rows),16) int64 and dists (len(rows),16) f32."""
    q = p1n[rows].astype(np.float32)                   # (R,3)
    d = (np.sum(q * q, 1, dtype=np.float32)[:, None]
         + np.sum(p2n * p2n, 1, dtype=np.float32)[None, :]
         - 2.0 * (q @ p2n.T.astype(np.float32)))
    d[:, len2:] = np.inf
    part = np.argpartition(d, K - 1, axis=1)[:, :K]
    pv = np.take_along_axis(d, part, axis=1)
    ordr = np.lexsort((part, pv), axis=1)
    j = np.take_along_axis(part, ordr, axis=1)
    return j.astype(np.int64), np.take_along_axis(pv, ordr, axis=1)


def kernel(p1, p2, lengths1, lengths2):
    from concourse.bass_utils import run_bass_kernel_spmd

    p1 = np.asarray(p1, np.float32)
    p2 = np.asarray(p2, np.float32)
    lengths1 = np.asarray(lengths1, np.int32)
    lengths2 = np.asarray(lengths2, np.int32)

    nc = _build_program()
    in_maps = [_core_inputs(p1, p2, lengths2, c) for c in range(N_CORES)]
    res = run_bass_kernel_spmd(nc, in_maps, core_ids=list(range(N_CORES)))

    # host epilogue: final top-16 of 64 candidates, decode indices, exact
    # dists via gather; exactness fallback for windows that may have held >8
    # of the true top-16.
    p1sq = np.sum(p1 * p1, axis=2, dtype=np.float32)    # (4, 8192)
    p2sq = np.sum(p2 * p2, axis=2, dtype=np.float32)    # (4, 8192)

    dists = np.zeros((N, P1, K), np.float32)
    idx = np.zeros((N, P1, K), np.int64)
    for c in range(N_CORES):
        n, h = c // 2, c % 2
        sl = slice(h * QPC, (h + 1) * QPC)
        # query-major candidate arrays: (QPC, 64)
        cv = (res.results[c]["cv_out"].reshape(TILE, NTILES, NCAND)
              .transpose(1, 0, 2).reshape(QPC, NCAND))
        ci = (res.results[c]["cidx_out"].reshape(TILE, NTILES, NCAND)
              .transpose(1, 0, 2).reshape(QPC, NCAND))
        # final top-16 of the candidates (desc value, ties -> lower pos)
        part = np.argpartition(-cv, K - 1, axis=1)[:, :K]
        pv = np.take_along_axis(cv, part, axis=1)
        ordr = np.lexsort((part, -pv), axis=1)
        pos = np.take_along_axis(part, ordr, axis=1)     # (QPC, 16) cand pos
        local = np.take_along_axis(ci, pos, axis=1).astype(np.int64)
        j = (pos >> 3) * WIN + local                     # global p2 indices
        # exact dists via gather
        g = p2[n][j]                                     # (4096, 16, 3)
        dots = np.einsum("pd,pkd->pk", p1[n, sl], g)
        dists[n, sl] = p1sq[n, sl, None] + p2sq[n][j] - 2.0 * dots
        idx[n, sl] = j
        # fallback: rows where one window contributed all 8 of its candidates
        win_counts = np.zeros((QPC, NWIN), np.int8)
        np.add.at(win_counts, (np.arange(QPC)[:, None], pos >> 3), 1)
        bad = np.nonzero(win_counts.max(axis=1) >= 8)[0]
        if bad.size:
            jb, db = _exact_rows(p1[n, sl], p2[n], int(lengths2[n]), bad)
            idx[n, h * QPC + bad] = jb
            dists[n, h * QPC + bad] = db

    for n in range(N):
        L = int(lengths1[n])
        dists[n, L:] = 0.0
        idx[n, L:] = 0
    return idx, dists


# revision 8
# speedup vs baseline: 1.3896x; 1.2688x over previous
"""KNN top-16 kernel for Trainium2 (8 NeuronCores, SPMD) — v4.

Problem (hardcoded shapes): p1 (4,8192,3) f32, p2 (4,8192,3) f32,
lengths1/2 (4,) i32.  Returns (idx int64 (4,8192,16), dists f32 (4,8192,16))
matching jax.lax.top_k(-sq_dists, 16) semantics with PyTorch3D-style padding.

Device algorithm (per work unit = one 128-query tile x one 1024-col window):
  score s[i,j] = 2*p1_i.p2_j - ||p2_j||^2 - BIG*(j >= len2), via a single
  22-contraction-row bf16 matmul (3-way bf16 split of both operands: 6 cross
  terms per dim + 3 rows for the fp32-split ||p2||^2 + 1 mask row) —
  ~fp32-accurate but 4x faster PE streaming than fp32.  The DVE reads the
  2-bank PSUM window directly: max8 -> top-8 values, max_index -> local
  indices.  Host merges per-window candidates, takes top-16 (ties: lower
  index first, like lax.top_k), and recomputes dists exactly by gathering.

Load balancing: query rows i >= lengths1[n] are zeroed by the reference and
columns j >= lengths2[n] are masked out, so only
ceil(len1/128) x ceil(len2/1024) units per batch carry information.  Work
units are packed into groups (one rhs window shared by R=4 query tiles) and
distributed evenly across the 8 cores; the host packs each core's input so
the (SPMD-uniform) program only touches useful data.  Unused slots hold
zeros and are dropped at decode.

Exactness guard: a window's top-8 cannot cover >8 of a row's true top-16;
any row where one window contributes all 8 of its candidates to the final
top-16 *might* have lost a 9th - such rows are recomputed exactly on the
host (the detection is complete, so the merged result is exact up to
matmul rounding ~1e-7).
"""

import numpy as np
from functools import lru_cache

N, P1, P2, D, K = 4, 8192, 8192, 3, 16
N_CORES = 8
TILE = 128             # query rows per tile
WIN = 1024             # top-8 window width == one 2-bank PSUM tile
MMN = 512              # matmul free-dim per instruction
ROWS = 22              # contraction rows
R = 4                  # query tiles per group (share one rhs window)
BIG = np.float32(1e30)


def _plan(lengths1, lengths2):
    """Pack useful (batch, qtile, window) units into per-core group lists.

    Returns (G, cores) where cores[c] is a list of G entries, each
    (n, w, qts) with qts a tuple of R qtile indices (-1 = unused slot),
    or None for an all-dummy group.
    """
    groups = []
    for n in range(N):
        nqt = max(1, min(P1 // TILE, -(-int(lengths1[n]) // TILE)))
        nw = max(1, min(P2 // WIN, -(-int(lengths2[n]) // WIN)))
        for w in range(nw):
            for a in range(0, nqt, R):
                qts = tuple(range(a, min(a + R, nqt)))
                qts = qts + (-1,) * (R - len(qts))
                groups.append((n, w, qts))
    G = -(-len(groups) // N_CORES)
    groups += [None] * (N_CORES * G - len(groups))
    cores = [groups[c * G:(c + 1) * G] for c in range(N_CORES)]
    return G, cores


@lru_cache(maxsize=4)
def _build_program(G):
    from concourse.bass import Bass
    from concourse.tile import TileContext
    import concourse.mybir as mybir

    f32 = mybir.dt.float32
    bf16 = mybir.dt.bfloat16
    u32 = mybir.dt.uint32

    S = G * R                      # slots per core
    LW = S * TILE                  # lhs width (cols)
    RW = G * WIN                   # rhs width (cols)

    nc = Bass("TRN2", num_devices=N_CORES)

    inp_d = nc.dram_tensor("inp", [ROWS, LW + RW], bf16, kind="ExternalInput")
    cv_d = nc.dram_tensor("cv_out", [TILE, S * 8], f32, kind="ExternalOutput")
    cidx_d = nc.dram_tensor("cidx_out", [TILE, S * 8], u32,
                            kind="ExternalOutput")

    with TileContext(nc) as tc:
        with tc.tile_pool(name="const", bufs=1) as cpool, \
             tc.tile_pool(name="psum", bufs=4, space="PSUM") as ppool:
            inp_sb = cpool.tile([ROWS, LW + RW], bf16)
            # piecewise input DMA, interleaved lhs/rhs so early groups land
            # first and compute overlaps the remaining transfer
            NPIECE = 8
            lsz = -(-S // NPIECE) * TILE
            rsz = -(-G // NPIECE) * WIN
            for i in range(NPIECE):
                la, lb = i * lsz, min((i + 1) * lsz, LW)
                ra, rb = LW + i * rsz, min(LW + (i + 1) * rsz, LW + RW)
                if la < lb:
                    nc.sync.dma_start(inp_sb[:, la:lb], inp_d[:, la:lb])
                if ra < rb:
                    nc.sync.dma_start(inp_sb[:, ra:rb], inp_d[:, ra:rb])
            lhs_sb = inp_sb[:, 0:LW]
            rhs_sb = inp_sb[:, LW:LW + RW]

            # Persistent result staging: each region written exactly once, so
            # DVE writes carry no slot-reuse deps.
            cand_v = cpool.tile([TILE, S * 8], f32)
            cand_i = cpool.tile([TILE, S * 8], u32)

            OUT_SPLIT = 4
            gpp = -(-G // OUT_SPLIT)   # groups per output piece
            for g in range(G):
                rhs_g = rhs_sb[:, g * WIN:(g + 1) * WIN]
                for r in range(R):
                    s = g * R + r
                    lhsT = lhs_sb[:, s * TILE:(s + 1) * TILE]
                    ps = ppool.tile([TILE, WIN], f32, tag="ps")
                    for h in range(WIN // MMN):
                        nc.tensor.matmul(
                            ps[:, h * MMN:(h + 1) * MMN], lhsT,
                            rhs_g[:, h * MMN:(h + 1) * MMN],
                            start=True, stop=True,
                        )
                    cv = cand_v[:, s * 8:(s + 1) * 8]
                    nc.vector.max(out=cv, in_=ps)
                    nc.vector.max_index(
                        out=cand_i[:, s * 8:(s + 1) * 8], in_max=cv,
                        in_values=ps)
                if (g + 1) % gpp == 0 or g == G - 1:
                    a = (g // gpp) * gpp * R * 8
                    b = (g + 1) * R * 8
                    nc.sync.dma_start(cv_d[:, a:b], cand_v[:, a:b])
                    nc.sync.dma_start(cidx_d[:, a:b], cand_i[:, a:b])

    # This walrus build allows only ~1 sync wait per instruction; split all
    # but the last wait onto single-wait NoOps chained before it (same
    # engine, program order => identical blocking semantics).
    import concourse.mybir as mb
    fix = 0
    for fn in nc.m.functions:
        for blk in fn.blocks:
            insts = blk.instructions
            i = 0
            while i < len(insts):
                inst = insts[i]
                si = inst.sync_info
                if si is not None and len(si.on_wait) > 1:
                    head, last = si.on_wait[:-1], si.on_wait[-1:]
                    pre = []
                    for w in head:
                        fix += 1
                        nop = mb.InstNoOp(name=f"I-waitfix-{fix}", ins=[],
                                          outs=[])
                        nop.engine = inst.engine
                        nop.sync_info = mb.SyncInfo(on_wait=[w], on_update=[])
                        pre.append(nop)
                    si.on_wait = last
                    insts[i:i] = pre
                    i += len(pre)
                i += 1
    return nc


def _split3(x):
    """3-way bf16 split: x ~= h + l1 + l2 (all bf16), error ~2^-27 |x|."""
    import ml_dtypes
    bf = ml_dtypes.bfloat16
    x = np.asarray(x, np.float32)
    h = x.astype(bf)
    l1 = (x - h.astype(np.float32)).astype(bf)
    l2 = (x - h.astype(np.float32) - l1.astype(np.float32)).astype(bf)
    return h, l1, l2


def _batch_rows(p1n, p2n, len2):
    """Full-width LHS [22, P1] and RHS [22, P2] bf16 contraction rows for one
    batch; slots/windows are column slices of these."""
    import ml_dtypes
    bf = ml_dtypes.bfloat16
    L = np.zeros((ROWS, P1), bf)
    Rm = np.zeros((ROWS, P2), bf)
    r = 0
    for d in range(D):
        A0, A1, A2 = _split3(p1n[:, d])
        B0, B1, B2 = _split3(p2n[:, d])
        A0f, A1f, A2f = (a.astype(np.float32) for a in (A0, A1, A2))
        # lhs carries the 2x (exact in bf16: *2 bumps the exponent)
        for a, b in [(A0f, B0), (A0f, B1), (A0f, B2),
                     (A1f, B0), (A2f, B0), (A1f, B1)]:
            L[r] = (2.0 * a).astype(bf)
            Rm[r] = b
            r += 1
    p2sq = (p2n.astype(np.float32) ** 2).sum(axis=1, dtype=np.float32)
    for s in _split3(p2sq):
        L[r] = bf(-1.0)
        Rm[r] = s
        r += 1
    L[r] = bf(1.0)
    Rm[r] = np.where(np.arange(P2) >= len2, -BIG, np.float32(0.0)).astype(bf)
    r += 1
    assert r == ROWS
    return L, Rm


def _core_inputs(batch_L, batch_R, core_groups, G):
    import ml_dtypes
    bf = ml_dtypes.bfloat16
    S = G * R
    LW, RW = S * TILE, G * WIN
    inp = np.zeros((ROWS, LW + RW), bf)
    for g, entry in enumerate(core_groups):
        if entry is None:
            continue
        n, w, qts = entry
        inp[:, LW + g * WIN:LW + (g + 1) * WIN] = \
            batch_R[n][:, w * WIN:(w + 1) * WIN]
        for r, qt in enumerate(qts):
            if qt < 0:
                continue
            s = g * R + r
            inp[:, s * TILE:(s + 1) * TILE] = \
                batch_L[n][:, qt * TILE:(qt + 1) * TILE]
    return {"inp": inp}


def _exact_rows(p1n, p2n, len2, rows):
    """Exact top-16 (reference fp32 formula) for the given query rows of one
    batch. Returns idx (len(rows),16) int64 and dists (len(rows),16) f32."""
    q = p1n[rows].astype(np.float32)                   # (Rn,3)
    d = (np.sum(q * q, 1, dtype=np.float32)[:, None]
         + np.sum(p2n * p2n, 1, dtype=np.float32)[None, :]
         - 2.0 * (q @ p2n.T.astype(np.float32)))
    d[:, len2:] = np.inf
    part = np.argpartition(d, K - 1, axis=1)[:, :K]
    pv = np.take_along_axis(d, part, axis=1)
    ordr = np.lexsort((part, pv), axis=1)
    j = np.take_along_axis(part, ordr, axis=1)
    return j.astype(np.int64), np.take_along_axis(pv, ordr, axis=1)


def kernel(p1, p2, lengths1, lengths2):
    from concourse.bass_utils import run_bass_kernel_spmd

    p1 = np.asarray(p1, np.float32)
    p2 = np.asarray(p2, np.float32)
    lengths1 = np.asarray(lengths1, np.int32)
    lengths2 = np.asarray(lengths2, np.int32)

    G, cores = _plan(lengths1, lengths2)
    nc = _build_program(G)
    batch_L = {}
    batch_R = {}
    for n in range(N):
        batch_L[n], batch_R[n] = _batch_rows(p1[n], p2[n], int(lengths2[n]))
    in_maps = [_core_inputs(batch_L, batch_R, cores[c], G)
               for c in range(N_CORES)]
    res = run_bass_kernel_spmd(nc, in_maps, core_ids=list(range(N_CORES)))

    p1sq = np.sum(p1 * p1, axis=2, dtype=np.float32)    # (4, 8192)
    p2sq = np.sum(p2 * p2, axis=2, dtype=np.float32)    # (4, 8192)

    dists = np.zeros((N, P1, K), np.float32)
    idx = np.zeros((N, P1, K), np.int64)

    # scatter per-core candidates into per-batch (row, window) tables
    NWMAX = P2 // WIN
    cv_b = [np.full((P1, NWMAX * 8), -np.inf, np.float32) for _ in range(N)]
    jb_b = [np.zeros((P1, NWMAX * 8), np.int64) for _ in range(N)]
    for c in range(N_CORES):
        cv = res.results[c]["cv_out"]                   # (128, S*8)
        ci = res.results[c]["cidx_out"].astype(np.int64)
        for g, entry in enumerate(cores[c]):
            if entry is None:
                continue
            n, w, qts = entry
            for r, qt in enumerate(qts):
                if qt < 0:
                    continue
                s = g * R + r
                rows = slice(qt * TILE, (qt + 1) * TILE)
                cols = slice(w * 8, (w + 1) * 8)
                cv_b[n][rows, cols] = cv[:, s * 8:(s + 1) * 8]
                jb_b[n][rows, cols] = w * WIN + ci[:, s * 8:(s + 1) * 8]

    for n in range(N):
        L1 = int(lengths1[n])
        nrows = min(P1, -(-L1 // TILE) * TILE)
        cvn = cv_b[n][:nrows]
        jbn = jb_b[n][:nrows]
        part = np.argpartition(-cvn, K - 1, axis=1)[:, :K]
        pv = np.take_along_axis(cvn, part, axis=1)
        pj = np.take_along_axis(jbn, part, axis=1)
        # ties: lower p2 index first (lax.top_k order)
        ordr = np.lexsort((pj, -pv), axis=1)
        j = np.take_along_axis(pj, ordr, axis=1)        # (nrows, 16)
        pos = np.take_along_axis(part, ordr, axis=1)
        # exact dists via gather (reference fp32 formula)
        g2 = p2[n][j]                                   # (nrows, 16, 3)
        dots = np.einsum("pd,pkd->pk", p1[n, :nrows], g2)
        dists[n, :nrows] = p1sq[n, :nrows, None] + p2sq[n][j] - 2.0 * dots
        idx[n, :nrows] = j
        # fallback: rows where one window contributed all 8 of its candidates
        win_counts = np.zeros((nrows, NWMAX), np.int8)
        np.add.at(win_counts, (np.arange(nrows)[:, None], pos >> 3), 1)
        bad = np.nonzero(win_counts.max(axis=1) >= 8)[0]
        if bad.size:
            jx, dx = _exact_rows(p1[n], p2[n], int(lengths2[n]), bad)
            idx[n, bad] = jx
            dists[n, bad] = dx
        dists[n, L1:] = 0.0
        idx[n, L1:] = 0
    return idx, dists
